# revision 1
# baseline (speedup 1.0000x reference)
"""Trainium2 Bass kernel for single-head attention.

reference:
  q = x @ Wq.T ; k = x @ Wk.T ; v = x @ Wv.T        (x: [B,S,D], W*: [D,D])
  out = softmax(q @ k.T / sqrt(D)) @ v              (B=4, S=4096, D=256)

Sharding: 8 cores = (batch b in 0..3) x (query-half h in 0..1).
Each core receives x^T for its batch, columns permuted so its 2048 queries
are columns 0:2048 (attention is permutation-invariant over keys, so K/V
built from the permuted sequence give identical results).  Host passes
transposed inputs (x^T, Wq^T, Wk^T, Wv^T) so the device does no layout
transposes.

Each core computes (fp32r matmuls):
  K^T [256,4096], Q^T [256,2048], V [4096,256]
then a flash-style pass over 128-key chunks:
  S^T = K_chunk @ Q^T  -> exp(S^T/16) = P^T (ACT; no max subtraction: scores
  are ~N(0,1) so exp cannot overflow in fp32)
  O^T += V_chunk.T @ P^T  (PE) ;  pacc += P^T  (DVE, elementwise)
  sums = ones.T @ pacc (replicated on all rows) ; out = O^T * (1/sums)
Core output is O^T [256, 2048]; the host transposes and scatters.
"""

from contextlib import ExitStack

import numpy as np

B, S, D = 4, 4096, 256
H = S // 2          # queries per core
NCORE = 8
KC = S // 128       # 32 key chunks
QT = H // 512       # 4 query tiles
SCALE = 1.0 / np.sqrt(D)

_compiled_nc = None


def _build():
    import concourse.mybir as mybir
    import concourse.tile as tile
    from concourse import bacc

    F32 = mybir.dt.float32
    FR = mybir.dt.float32r
    EXP = mybir.ActivationFunctionType.Exp

    nc = bacc.Bacc("TRN2", target_bir_lowering=False, debug=False, num_devices=NCORE)
    xt = nc.dram_tensor("xt", [D, S], F32, kind="ExternalInput")
    wqt_d = nc.dram_tensor("wqt", [D, D], F32, kind="ExternalInput")
    wvt_d = nc.dram_tensor("wvt", [D, D], F32, kind="ExternalInput")
    ot = nc.dram_tensor("ot", [D, H], F32, kind="ExternalOutput")

    with tile.TileContext(nc) as tc, ExitStack() as ctx:
        const = ctx.enter_context(tc.tile_pool(name="const", bufs=1))
        big = ctx.enter_context(tc.tile_pool(name="big", bufs=1))
        pt_pool = ctx.enter_context(tc.tile_pool(name="ptp", bufs=6))
        small = ctx.enter_context(tc.tile_pool(name="small", bufs=2))

        _cp_flip = [0]

        def copy_out(dst, srcap):
            # alternate PSUM->SBUF evacuation between DVE and ACT
            _cp_flip[0] ^= 1
            if _cp_flip[0]:
                nc.vector.tensor_copy(dst, srcap)
            else:
                nc.scalar.copy(dst, srcap)

        ones_f = const.tile([128, 128], F32, name="ones_f")
        nc.vector.memset(ones_f, 1.0)
        ones_r = const.tile([128, 128], FR, name="ones_r")
        nc.vector.tensor_copy(ones_r, ones_f)

        # pre-transposed weights: w*t [128, dc, a] = W.T[dc*128 + p, a]
        # wqt now holds G^T = Wq^T @ Wk (host-computed), so Y = G^T.T @ x^T
        wqt = const.tile([128, 2, 256], FR, name="wqt")
        wvt = const.tile([128, 2, 256], FR, name="wvt")
        for dst, src in ((wqt, wqt_d), (wvt, wvt_d)):
            nc.gpsimd.dma_start(dst, src[:, :].rearrange("(c p) a -> p c a", p=128).bitcast(FR))

        # persistent tensors
        xT = big.tile([128, 2, KC, 128], FR, name="xT")
        # Y = (Wk^T Wq) @ x^T  [d, q] -- S^T = x^T_chunk.T @ Y (K and Q never built)
        yt = big.tile([128, 2, QT, 512], FR, name="yt")
        vt = big.tile([128, KC, 256], FR, name="vt")
        osb = [big.tile([128, QT, 512], F32, name=f"osb{ec}") for ec in range(2)]

        # x^T load: [256, 4096] -> [128 part, 2 dc, 32 block, 128], chunked DMAs
        # (smaller leading chunks so the first projections can start earlier)
        xt_r = xt[:, :].rearrange("(c p) (n f) -> p c n f", p=128, f=128).bitcast(FR)
        edges = [0, 2, 4, 8, 16, 24, 32]
        for c in range(len(edges) - 1):
            sl = slice(edges[c], edges[c + 1])
            nc.sync.dma_start(xT[:, :, sl, :], xt_r[:, :, sl, :])

        # ---- phase 1: project K/Q/V, chunk-pipelined with the x^T DMAs ----
        with ExitStack() as p1:
            pj_pool = p1.enter_context(tc.tile_pool(name="pj_psum", bufs=4, space="PSUM"))
            pv_pool = p1.enter_context(tc.tile_pool(name="pv_psum", bufs=4, space="PSUM"))

            for g2 in range(8):
                # Y[:, dc, g2, :] = sum_e G[d, e] x^T[e, q]  (q tiles live in blocks 0..15)
                if g2 < 4:
                    for dc in range(2):
                        py = pj_pool.tile([128, 512], F32, tag="pj", name=f"py{dc}{g2}")
                        nc.tensor.matmul(py, wqt[:, 0, dc * 128:(dc + 1) * 128], xT[:, 0, g2 * 4:(g2 + 1) * 4, :], start=True, stop=False)
                        nc.tensor.matmul(py, wqt[:, 1, dc * 128:(dc + 1) * 128], xT[:, 1, g2 * 4:(g2 + 1) * 4, :], start=False, stop=True)
                        copy_out(yt[:, dc, g2, :], py)
                # V for these 4 blocks
                for nb in range(4):
                    n = g2 * 4 + nb
                    pv = pv_pool.tile([128, 256], F32, tag="pv", name=f"pv{n}")
                    nc.tensor.matmul(pv, xT[:, 0, n, :], wvt[:, 0, :], start=True, stop=False)
                    nc.tensor.matmul(pv, xT[:, 1, n, :], wvt[:, 1, :], start=False, stop=True)
                    copy_out(vt[:, n, :], pv)

        # ---- phase 2: flash attention over key chunks ----
        with ExitStack() as p2:
            st_pool = p2.enter_context(tc.tile_pool(name="st_psum", bufs=2, space="PSUM"))
            acc_pool = p2.enter_context(tc.tile_pool(name="acc_psum", bufs=1, space="PSUM"))

            for j in range(QT):
                ot0 = acc_pool.tile([128, 512], F32, tag="ot0", name=f"ot0_{j}")
                ot1 = acc_pool.tile([128, 512], F32, tag="ot1", name=f"ot1_{j}")
                pacc = small.tile([128, 2, 512], FR, tag="pacc", name=f"pacc{j}")
                for g in range(KC // 2):
                    st = st_pool.tile([128, 2, 512], F32, tag="st", name=f"st{j}_{g}")
                    for u in range(2):
                        kc = g * 2 + u
                        nc.tensor.matmul(st[:, u, :], xT[:, 0, kc, :], yt[:, 0, j, :], start=True, stop=False)
                        nc.tensor.matmul(st[:, u, :], xT[:, 1, kc, :], yt[:, 1, j, :], start=False, stop=True)
                    pt = pt_pool.tile([128, 2, 512], FR, tag="pt", name=f"pt{j}_{g}")
                    nc.scalar.activation(pt, st, EXP, scale=float(SCALE))
                    # accumulate exp tiles elementwise on DVE (softmax denominator:
                    # cross-partition sum happens once at the end via ones-matmul)
                    if g == 0:
                        nc.vector.tensor_copy(pacc, pt)
                    else:
                        nc.vector.tensor_add(pacc, pacc, pt)
                    for u in range(2):
                        kc = g * 2 + u
                        first, last = kc == 0, kc == KC - 1
                        nc.tensor.matmul(ot0, vt[:, kc, 0:128], pt[:, u, :], start=first, stop=last)
                        nc.tensor.matmul(ot1, vt[:, kc, 128:256], pt[:, u, :], start=first, stop=last)
                # softmax denominator
                smt = acc_pool.tile([128, 512], F32, tag="sm", name=f"smt{j}")
                sm = smt[:, :]
                for u in range(2):
                    nc.tensor.matmul(sm, ones_r, pacc[:, u, :], start=(u == 0), stop=(u == 1))
                rc = small.tile([128, 512], F32, tag="rc", name=f"rc{j}")
                nc.vector.reciprocal_approx_fast(rc, sm)
                for ec, acc in ((0, ot0), (1, ot1)):
                    for hh in range(2):
                        sl = slice(hh * 256, (hh + 1) * 256)
                        nc.vector.tensor_mul(osb[ec][:, j, sl], acc[:, sl], rc[:, sl])
                        nc.sync.dma_start(
                            ot[ec * 128:(ec + 1) * 128, j * 512 + hh * 256:j * 512 + (hh + 1) * 256],
                            osb[ec][:, j, sl],
                        )

    nc.compile()
    return nc


def _get_nc():
    global _compiled_nc
    if _compiled_nc is None:
        _compiled_nc = _build()
    return _compiled_nc


def make_in_maps(x, Wq, Wk, Wv):
    x = np.asarray(x, dtype=np.float32)
    gT = np.ascontiguousarray(
        (np.asarray(Wq, dtype=np.float64).T @ np.asarray(Wk, dtype=np.float64)).astype(np.float32))
    wvT = np.ascontiguousarray(np.asarray(Wv, dtype=np.float32).T)
    in_maps = []
    for c in range(NCORE):
        b, h = c // 2, c % 2
        xb = x[b]
        if h == 1:
            xb = np.concatenate([xb[H:], xb[:H]], axis=0)
        in_maps.append({
            "xt": np.ascontiguousarray(xb.T),
            "wqt": gT,
            "wvt": wvT,
        })
    return in_maps


def kernel(x, Wq, Wk, Wv):
    from concourse.bass_utils import run_bass_kernel_spmd

    nc = _get_nc()
    in_maps = make_in_maps(x, Wq, Wk, Wv)
    res = run_bass_kernel_spmd(nc, in_maps, core_ids=list(range(NCORE)))
    out = np.empty((B, S, D), dtype=np.float32)
    for c in range(NCORE):
        b, h = c // 2, c % 2
        out[b, h * H:(h + 1) * H, :] = res.results[c]["ot"].T
    return out



# revision 2
# speedup vs baseline: 1.3248x; 1.3248x over previous
"""Trainium2 Bass kernel for single-head attention.

reference:
  q = x @ Wq.T ; k = x @ Wk.T ; v = x @ Wv.T        (x: [B,S,D], W*: [D,D])
  out = softmax(q @ k.T / sqrt(D)) @ v              (B=4, S=4096, D=256)

Sharding: 8 cores = (batch b in 0..3) x (query-half h in 0..1).
Each core receives x^T for its batch in fp16, columns permuted so its 2048
queries are columns 0:2048 (attention is permutation-invariant over keys, so
K/V built from the permuted sequence give identical results).

All matmul operands are fp16 (fp32 PSUM accumulation): the PE streams fp16 at
the same 1 col/cycle as fp32r, but fp16 weights get Fast-Weight-Load, halving
the LDWEIGHTS cost that dominated the fp32r version.  Scores fold Wq/Wk into
G = Wq^T Wk host-side (q.k = x_q G x_k^T), so only two projections run on
device:
  Y[a,q] = sum_e G[e,a] x^T[e,q]      (G stationary: 4 weight loads total)
  V[k,e] = sum_d x^T[d,k] Wv^T[d,e]   (x chunk stationary)
Flash pass, 1024 queries at a time (jp=0,1), key chunks kc of 128:
  S^T[k,q] = sum_d x^T[d,k] Y[d,q]  -> exp(S^T/16) = P^T fp16 (ACT, 2x512)
  O^T[d,q] += V_chunk^T @ P^T (PE, fp32 PSUM) ; pacc += P^T (DVE fp16)
  sums = ones^T @ pacc ; out = O^T * (1/sums)
Scores for kc+1 are emitted before PV of kc so the PE never waits on the exp.
Core output is O^T [256, 2048] fp32; the host transposes and scatters.
"""

from contextlib import ExitStack

import numpy as np

B, S, D = 4, 4096, 256
H = S // 2          # queries per core
NCORE = 8
KC = S // 128       # 32 key chunks
SCALE = 1.0 / np.sqrt(D)

_compiled_nc = None


def _build():
    import concourse.mybir as mybir
    import concourse.tile as tile
    from concourse import bacc

    F16 = mybir.dt.float16
    F32 = mybir.dt.float32
    EXP = mybir.ActivationFunctionType.Exp

    nc = bacc.Bacc("TRN2", target_bir_lowering=False, debug=False, num_devices=NCORE)
    xt = nc.dram_tensor("xt", [D, S], F16, kind="ExternalInput")
    gt_d = nc.dram_tensor("gt", [D, D], F16, kind="ExternalInput")
    wvt_d = nc.dram_tensor("wvt", [D, D], F16, kind="ExternalInput")
    ot = nc.dram_tensor("ot", [D, H], F32, kind="ExternalOutput")

    with tile.TileContext(nc) as tc, ExitStack() as ctx:
        const = ctx.enter_context(tc.tile_pool(name="const", bufs=1))
        big = ctx.enter_context(tc.tile_pool(name="big", bufs=1))
        pt_pool = ctx.enter_context(tc.tile_pool(name="ptp", bufs=3))
        small = ctx.enter_context(tc.tile_pool(name="small", bufs=2))

        _cp_flip = [0]

        def copy_out(dst, srcap):
            # alternate PSUM->SBUF evacuation between DVE and ACT
            _cp_flip[0] ^= 1
            if _cp_flip[0]:
                nc.vector.tensor_copy(dst, srcap)
            else:
                nc.scalar.copy(dst, srcap)

        ones_f = const.tile([128, 128], F32, name="ones_f")
        nc.vector.memset(ones_f, 1.0)
        ones16 = const.tile([128, 128], F16, name="ones16")
        nc.vector.tensor_copy(ones16, ones_f)

        # g16[p, ec, a] = G[ec*128+p, a],  wv16[p, dc, e] = Wv^T[dc*128+p, e]
        g16 = const.tile([128, 2, 256], F16, name="g16")
        wv16 = const.tile([128, 2, 256], F16, name="wv16")
        for dst, src in ((g16, gt_d), (wv16, wvt_d)):
            nc.gpsimd.dma_start(dst, src[:, :].rearrange("(c p) a -> p c a", p=128))

        # persistent tensors
        xT = big.tile([128, 2, KC, 128], F16, name="xT")
        yt = big.tile([128, 2, 4, 512], F16, name="yt")
        vt = big.tile([128, KC, 256], F16, name="vt")

        # x^T load: [256, 4096] -> [128 part, 2 dc, 32 block, 128], chunked
        xt_r = xt[:, :].rearrange("(c p) (n f) -> p c n f", p=128, f=128)
        edges = [0, 4, 8, 16, 24, 32]
        for c in range(len(edges) - 1):
            sl = slice(edges[c], edges[c + 1])
            nc.sync.dma_start(xT[:, :, sl, :], xt_r[:, :, sl, :])

        # ---- phase 1: project Y and V, chunk-pipelined with the x^T DMAs ----
        with ExitStack() as p1:
            py_pool = p1.enter_context(tc.tile_pool(name="py_psum", bufs=1, space="PSUM"))
            pv_pool = p1.enter_context(tc.tile_pool(name="pv_psum", bufs=4, space="PSUM"))

            for ab in range(2):
                # Y[ab*128:(ab+1)*128, :] for all 2048 queries; G stationary
                py = py_pool.tile([128, 4, 512], F32, tag="py", name=f"py{ab}")
                for ec in range(2):
                    for g2 in range(4):
                        nc.tensor.matmul(
                            py[:, g2, :],
                            g16[:, ec, ab * 128:(ab + 1) * 128],
                            xT[:, ec, g2 * 4:(g2 + 1) * 4, :],
                            start=(ec == 0), stop=(ec == 1),
                        )
                for g2 in range(4):
                    copy_out(yt[:, ab, g2, :], py[:, g2, :])
                # V for 16 key blocks; x chunk stationary
                for nb in range(16):
                    n = ab * 16 + nb
                    pv = pv_pool.tile([128, 256], F32, tag="pv", name=f"pv{n}")
                    nc.tensor.matmul(pv, xT[:, 0, n, :], wv16[:, 0, :], start=True, stop=False)
                    nc.tensor.matmul(pv, xT[:, 1, n, :], wv16[:, 1, :], start=False, stop=True)
                    copy_out(vt[:, n, :], pv)

        # ---- phase 2: flash attention, 1024 queries per pass ----
        with ExitStack() as p2:
            st_pool = p2.enter_context(tc.tile_pool(name="st_psum", bufs=2, space="PSUM"))
            acc_pool = p2.enter_context(tc.tile_pool(name="acc_psum", bufs=1, space="PSUM"))

            for jp in range(2):
                otp = [acc_pool.tile([128, 2, 512], F32, tag=f"ot{dh}", name=f"ot{dh}_{jp}")
                       for dh in range(2)]
                pacc = small.tile([128, 2, 512], F16, tag="pacc", name=f"pacc{jp}")

                def emit_scores(kc):
                    st = st_pool.tile([128, 2, 512], F32, tag="st", name=f"st{jp}_{kc}")
                    for dc in range(2):
                        for qh in range(2):
                            nc.tensor.matmul(
                                st[:, qh, :],
                                xT[:, dc, kc, :],
                                yt[:, dc, 2 * jp + qh, :],
                                start=(dc == 0), stop=(dc == 1),
                            )
                    return st

                st_cur = emit_scores(0)
                for kc in range(KC):
                    st_next = emit_scores(kc + 1) if kc + 1 < KC else None
                    pt = pt_pool.tile([128, 2, 512], F16, tag="pt", name=f"pt{jp}_{kc}")
                    for qh in range(2):
                        nc.scalar.activation(pt[:, qh, :], st_cur[:, qh, :], EXP, scale=float(SCALE))
                    if kc == 0:
                        nc.vector.tensor_copy(pacc, pt)
                    else:
                        nc.vector.tensor_add(pacc, pacc, pt)
                    for dh in range(2):
                        for qh in range(2):
                            nc.tensor.matmul(
                                otp[dh][:, qh, :],
                                vt[:, kc, dh * 128:(dh + 1) * 128],
                                pt[:, qh, :],
                                start=(kc == 0), stop=(kc == KC - 1),
                            )
                    st_cur = st_next

                # softmax denominator (cross-partition sum via ones-matmul)
                smt = st_pool.tile([128, 2, 512], F32, tag="st", name=f"smt{jp}")
                for qh in range(2):
                    nc.tensor.matmul(smt[:, qh, :], ones16, pacc[:, qh, :], start=True, stop=True)
                rc = small.tile([128, 2, 512], F32, tag="rc", name=f"rc{jp}")
                nc.vector.reciprocal_approx_fast(rc, smt)
                for dh in range(2):
                    for qh in range(2):
                        osb = small.tile([128, 512], F32, tag="osb", name=f"osb{jp}{dh}{qh}")
                        nc.vector.tensor_mul(osb, otp[dh][:, qh, :], rc[:, qh, :])
                        nc.sync.dma_start(
                            ot[dh * 128:(dh + 1) * 128,
                               jp * 1024 + qh * 512:jp * 1024 + (qh + 1) * 512],
                            osb,
                        )

    nc.compile()
    return nc


def _get_nc():
    global _compiled_nc
    if _compiled_nc is None:
        _compiled_nc = _build()
    return _compiled_nc


def make_in_maps(x, Wq, Wk, Wv):
    x = np.asarray(x, dtype=np.float32)
    g = (np.asarray(Wq, dtype=np.float64).T @ np.asarray(Wk, dtype=np.float64))
    g16 = np.ascontiguousarray(g.astype(np.float16))
    wv16 = np.ascontiguousarray(np.asarray(Wv, dtype=np.float32).T.astype(np.float16))
    in_maps = []
    for c in range(NCORE):
        b, h = c // 2, c % 2
        xb = x[b]
        if h == 1:
            xb = np.concatenate([xb[H:], xb[:H]], axis=0)
        in_maps.append({
            "xt": np.ascontiguousarray(xb.T.astype(np.float16)),
            "gt": g16,
            "wvt": wv16,
        })
    return in_maps


def kernel(x, Wq, Wk, Wv):
    from concourse.bass_utils import run_bass_kernel_spmd

    nc = _get_nc()
    in_maps = make_in_maps(x, Wq, Wk, Wv)
    res = run_bass_kernel_spmd(nc, in_maps, core_ids=list(range(NCORE)))
    out = np.empty((B, S, D), dtype=np.float32)
    for c in range(NCORE):
        b, h = c // 2, c % 2
        out[b, h * H:(h + 1) * H, :] = res.results[c]["ot"].T
    return out
